# revision 8
# baseline (speedup 1.0000x reference)
"""Paged GQA attention Bass kernel for TRN2, SPMD over 8 cores (v6).

Sharding: tensor-parallel over KV heads. Core h owns KV head h and its 4
query heads. Per-core: B=4 seqs x S=2048 kv x (4 heads * 256 q) x d=128.

v6 design (HW-measured instruction rates; see work/micro.py):
  - host pre-scatters the new k/v tokens into the cache copy, so there is
    no device tail path. One pair-gather per 2 seqs covers all 128 real
    blocks: 16 slot-tiles per seq, kv position = 16j + c (permuted), no
    pad blobs, no den correction.
  - exp on ACT (616ns/tile measured, no bias needed for bf16 range);
    optionally a few tiles per seq on DVE via the custom ops EXP_SEED_ANT
    (2nd-order seed of exp(x*SCALE/64), 650ns) and EXP_FIN_MASK_ANT
    (^64 + causal compare vs qp, 635ns).
  - causal masks are dense bf16 [128,1024] consts (rows j<112 are ones),
    applied with tensor_mul (199ns measured); accumulation is
    tensor_copy/tensor_add (199ns). scalar_tensor_tensor measured 3.5x
    slower than tensor_tensor on HW (cost model is wrong there) - v5's
    mistake.
  - den: partial^T via one XBAR transpose -> tensor_reduce (axis X) ->
    [128, 8] f32 -> reciprocal. No PE ones-matmul, no den transposes.
  - output: psum_o -> osb bf16 (DVE) -> XBAR -> 4x bf16 scale ops -> f32
    cast on the output DMA (gpsimd).
  - PV stays bf16 (fp8 et/V measured 3.5e-2..6.8e-2 rel err vs the 2e-2
    gate). PV matmuls for DVE-exp'd tiles are deferred to the end of the
    seq's PE stream so PE never waits on the slower DVE exp.
  PE is the bottleneck: 32768 cycles/seq (scores+PV) ~ 54.6us/rep.
"""
import numpy as np
import ml_dtypes

import concourse.bass as bass
import concourse.bacc as bacc
import concourse.mybir as mybir
from concourse.tile import TileContext

F32 = mybir.dt.float32
BF16 = mybir.dt.bfloat16
I16 = mybir.dt.int16

B, Q, S = 4, 256, 2048
G, D = 4, 128
BLOCK = 16
NBLK = 640               # cache pool blocks
NT = 16                  # kv tiles per seq (slot-tiles)
QW = G * Q               # 1024
SCALE = float(D) ** -0.5
EXP_N = 64               # seed^64: 6 squarings
DVE_TILES_DEFAULT = (4, 9, 14)


# ---------------------------------------------------------------------------
# Custom DVE exp ops (registered into concourse.dve_ops on first use).
# ---------------------------------------------------------------------------
_EXP_OPS = {}


def _register_exp_ops():
    if _EXP_OPS:
        return _EXP_OPS
    import concourse.dve_ops as dve_ops
    from concourse.dve_spec import (
        Spec, Src0, Src1, C0, C1, C2, sq, lower, _has_src1,
    )
    from concourse.dve_uop import DveOpSpec

    def _seed_ref(in0, in1, s0, s1, imm2):
        return (imm2 * (in0.astype(np.float32) * s0 + s1) ** 2
                + imm2).astype(np.float32)

    seed_spec = Spec(
        body=sq(Src0 * C0 + C1) * C2 + C2,
        reference=_seed_ref,
    )

    x = Src0
    for _ in range(6):
        x = sq(x)
    fin_spec = Spec(
        body=x * (Src1 >= C0),
        reference=lambda in0, in1, s0, s1, imm2: (
            (in0.astype(np.float32) ** 64)
            * (in1.astype(np.float32) >= s0)
        ).astype(np.float32),
    )

    for name, spec in (("EXP_SEED_ANT", seed_spec),
                       ("EXP_FIN_MASK_ANT", fin_spec)):
        if name not in dve_ops._SUB_OPCODE_FOR_NAME:
            row = max(dve_ops._SUB_OPCODE_FOR_NAME.values()) + 1
            assert row < 0x20, "custom DVE opcode rows exhausted"
            dve_ops._SUB_OPCODE_FOR_NAME[name] = row
        row = dve_ops._SUB_OPCODE_FOR_NAME[name]
        shas = {}
        for ver in ("v3", "v4"):
            uops = lower(spec, ver=ver)
            shas[ver] = DveOpSpec(
                name=name, opcode=row, uops=uops, rd1_en=_has_src1(spec)
            ).sha(ver)
        op = dve_ops.DveOp(name, spec, subdim=False, uops_sha=shas)
        if all(o.name != name for o in dve_ops.OPS):
            dve_ops.OPS.append(op)
        dve_ops.CUSTOM_DVE_SPECS[name] = spec
        _EXP_OPS[name] = op
    return _EXP_OPS


def build_consts(seq_lens):
    """qp [128, QW] bf16: t*128+k per column (exact in bf16), used by the
    custom DVE mask compare. c0s [128, B*NT] f32: threshold
    16j + c - (sl_b - 256). masks: dense bf16 [128, QW] visibility tiles
    (deduped); needs[(b, c)] -> mask name or None (all-visible)."""
    col = np.arange(QW)
    tk = (col % Q).astype(np.float64)
    qp = np.broadcast_to(tk[None, :], (128, QW)).astype(ml_dtypes.bfloat16)
    j = np.arange(128)
    c0s = np.zeros((128, B * NT), np.float32)
    mask_arrays, needs, cache = {}, {}, {}
    for b in range(B):
        sl = int(seq_lens[b])
        qpos = sl - Q + (col % Q)
        for c in range(NT):
            c0s[:, b * NT + c] = 16 * j + c - (sl - Q)
            kpos = 16 * j + c
            vis = kpos[:, None] <= qpos[None, :]
            if vis.all():
                needs[(b, c)] = None
                continue
            key = vis.tobytes()
            if key not in cache:
                name = f"mask{len(cache)}"
                cache[key] = name
                mask_arrays[name] = vis.astype(ml_dtypes.bfloat16)
            needs[(b, c)] = cache[key]
    return {"qp": np.ascontiguousarray(qp), "c0s": c0s, **mask_arrays}, needs


def build_nc(seq_lens=(2048,) * B, variant="full", repeat=1,
             dve_tiles=DVE_TILES_DEFAULT):
    exp_ops = _register_exp_ops()
    nc = bacc.Bacc(None, target_bir_lowering=False, debug=False)

    consts_arrays, mask_needs = build_consts(seq_lens)

    q_ext = nc.declare_dram_parameter("q", [B * Q, G * D], F32, isOutput=False)
    kvc_ext = nc.declare_dram_parameter("kvc", [2, NBLK, BLOCK * D], F32,
                                        isOutput=False)
    btw_ext = nc.declare_dram_parameter("btw", [128, B * 8], I16, isOutput=False)
    idb_ext = nc.declare_dram_parameter("idb", [128, 128], BF16, isOutput=False)
    qp_ext = nc.declare_dram_parameter("qp", [128, QW], BF16, isOutput=False)
    c0s_ext = nc.declare_dram_parameter("c0s", [128, B * NT], F32,
                                        isOutput=False)
    mask_ext = {
        name: nc.declare_dram_parameter(name, [128, QW], BF16, isOutput=False)
        for name in consts_arrays if name.startswith("mask")
    }

    out_ext = nc.declare_dram_parameter("out", [B * Q, G * D], F32, isOutput=True)

    if variant == "nodve":
        dve_tiles = ()
    dve_tiles = tuple(dve_tiles)

    from contextlib import ExitStack

    with TileContext(nc) as tc, ExitStack() as stack:
        cpool = stack.enter_context(tc.tile_pool(name="consts", bufs=1))
        dpool = stack.enter_context(tc.tile_pool(name="dram", bufs=1, space="DRAM"))
        kvpool = stack.enter_context(tc.tile_pool(name="kvp", bufs=2))
        spool = stack.enter_context(tc.tile_pool(name="sbuf", bufs=3))
        idxpool = stack.enter_context(tc.tile_pool(name="idxp", bufs=2))
        et_pool = stack.enter_context(tc.tile_pool(name="et", bufs=8))
        sd_pool = stack.enter_context(tc.tile_pool(name="sd", bufs=2))
        ppool_sc = stack.enter_context(tc.tile_pool(name="psc", bufs=3, space="PSUM"))
        ppool_o = stack.enter_context(tc.tile_pool(name="po", bufs=1, space="PSUM"))

        # ---- constants ----
        idb = cpool.tile([128, 128], BF16, tag="idb")
        nc.sync.dma_start(out=idb[:], in_=idb_ext[:, :])
        qp = cpool.tile([128, QW], BF16, tag="qp")
        nc.sync.dma_start(out=qp[:], in_=qp_ext[:, :])
        c0s = cpool.tile([128, B * NT], F32, tag="c0s")
        nc.sync.dma_start(out=c0s[:], in_=c0s_ext[:, :])
        masks = {}
        for name in mask_ext:
            m = cpool.tile([128, QW], BF16, tag=name)
            nc.sync.dma_start(out=m[:], in_=mask_ext[name][:, :])
            masks[name] = m

        if variant == "noop":
            z = spool.tile([128, 128], F32, tag="outsb")
            nc.vector.memset(z[:], 0.0)
            nc.sync.dma_start(out=out_ext[0:128, 0:128], in_=z[:])

        # PE clock warm-up (HAM gate holds PE at 1.2 GHz until ~3.4us busy).
        if variant != "noop":
            for _w in range(28):
                warm = ppool_sc.tile([128, 128], F32, tag="psc", name="warm")
                nc.tensor.matmul(warm[:], lhsT=idb[:], rhs=idb[:],
                                 start=True, stop=True)

        # ---- one-time staging ----
        # kvb blob tensor: [K block (16x128) | V block] per block, bf16.
        kvb = dpool.tile([NBLK, 2 * BLOCK * D], BF16, tag="kvb")
        nc.gpsimd.dma_start(
            out=kvb[:, :].rearrange("b (k e) -> k b e", k=2, e=BLOCK * D),
            in_=kvc_ext[:, :, :],
        )
        # q staged bf16: [p=tok%128, r=tok//128, (h d)]
        qcb = cpool.tile([128, (B * Q // 128) * G * D], BF16, tag="qcb")
        qcb_v = qcb[:].rearrange("p (r hd) -> p r hd", r=B * Q // 128, hd=G * D)
        nc.gpsimd.dma_start(
            out=qcb_v[:, :, :],
            in_=q_ext.rearrange("(r p) hd -> p r hd", p=128),
        )

        def emit_prep_dma(b, btwsb):
            """Pair gather for seqs (b, b+1): 256 idxs, kvt [128, 32, 256]."""
            st = {}
            nj = 256
            st["kvt"] = kvpool.tile([128, 32 * nj], BF16, tag="kvt", name="kvt")
            nc.gpsimd.dma_gather(
                out_ap=st["kvt"][:].rearrange("p (c j) -> p c j", c=32, j=nj),
                in_ap=kvb[:, :],
                idxs_ap=btwsb[:, b * 8 : b * 8 + nj // 16],
                num_idxs=nj, num_idxs_reg=nj, elem_size=2 * BLOCK * D,
                transpose=True, single_packet=False,
            )
            return st

        def emit_prep_compute(b, st, pair_side):
            nj = 256
            kvt_v = st["kvt"][:].rearrange("p (c j) -> p c j", c=32, j=nj)
            sd = pair_side
            if sd == 0:
                vtbp = kvpool.tile([128, 2 * NT * D], BF16, tag="vtb")
                nc.sync.dma_start_transpose(
                    out=vtbp[:].rearrange("p (m d) -> p m d", m=2 * NT, d=D),
                    in_=st["kvt"][:, 16 * nj : 32 * nj],
                )
                st["vtbp"] = vtbp
            vtb_v = st["vtbp"][:].rearrange(
                "p (c s d) -> p c s d", c=NT, s=2, d=D)
            st["kt_tiles"] = [kvt_v[:, i, sd * 128 : (sd + 1) * 128]
                              for i in range(NT)]
            st["v_tiles"] = [vtb_v[:, i, sd, :] for i in range(NT)]
            # q^T: [128 d, (h, t, tok)] via 2 XBAR transposes
            qt_t = spool.tile([128, QW], BF16, tag="qt")
            qt_v = qt_t[:].rearrange("p (h t k) -> p h t k", h=G, t=2, k=128)
            for t in range(2):
                nc.sync.dma_start_transpose(
                    out=qt_v[:, :, t, :],
                    in_=qcb_v[:, 2 * b + t, :],
                )
            st["qt"] = qt_t

        def emit_compute(b, st, mid_hook=None):
            partial = spool.tile([128, QW], BF16, tag="partial")
            psum_o = ppool_o.tile([128, QW], F32, tag="po")
            qt_t = st["qt"]
            kt_tiles, v_tiles = st["kt_tiles"], st["v_tiles"]

            pv_order = [i for i in range(NT) if i not in dve_tiles] + \
                       [i for i in range(NT) if i in dve_tiles]
            pv_last_tile = pv_order[-1]

            def emit_pv(i, et):
                if variant == "nopv":
                    return
                v_tile = v_tiles[i]
                for half in range(2):
                    nc.tensor.matmul(
                        psum_o[:, half * 512 : (half + 1) * 512],
                        lhsT=v_tile,
                        rhs=et[:, half * 512 : (half + 1) * 512],
                        start=(i == pv_order[0]), stop=(i == pv_last_tile),
                    )

            ets = {}
            pv_queue = []
            for i in range(NT):
                if i == 6 and mid_hook is not None:
                    mid_hook()
                psc = ppool_sc.tile([128, QW], F32, tag="psc")
                for half in range(2):
                    nc.tensor.matmul(
                        psc[:, half * 512 : (half + 1) * 512],
                        lhsT=kt_tiles[i],
                        rhs=qt_t[:, half * 512 : (half + 1) * 512],
                        start=True, stop=True,
                    )
                et = et_pool.tile([128, QW], BF16, tag="et")
                if i in dve_tiles:
                    c0 = c0s[:, b * NT + i : b * NT + i + 1]
                    sd1 = sd_pool.tile([128, QW], F32, tag="sd")
                    nc.vector._custom_dve(
                        exp_ops["EXP_SEED_ANT"], out=sd1[:], in0=psc[:],
                        s0=SCALE / EXP_N, s1=1.0, imm2=0.5,
                    )
                    nc.vector._custom_dve(
                        exp_ops["EXP_FIN_MASK_ANT"], out=et[:], in0=sd1[:],
                        in1=qp[:], s0=c0,
                    )
                else:
                    if variant == "noexp":
                        nc.scalar.activation(
                            et[:, 0:128], psc[:, 0:128],
                            mybir.ActivationFunctionType.Exp, scale=SCALE,
                        )
                    else:
                        nc.scalar.activation(
                            et[:], psc[:], mybir.ActivationFunctionType.Exp,
                            scale=SCALE,
                        )
                    mname = mask_needs[(b, i)]
                    if mname is not None and variant != "nomask":
                        nc.vector.tensor_mul(et[:], et[:], masks[mname][:])
                if variant != "noacc":
                    if i == 0:
                        nc.vector.tensor_copy(partial[:], et[:])
                    else:
                        nc.vector.tensor_add(partial[:], partial[:], et[:])
                ets[i] = et
                if i not in dve_tiles:
                    pv_queue.append(i)
                    if len(pv_queue) > 1:
                        j = pv_queue.pop(0)
                        emit_pv(j, ets.pop(j))
            for j in pv_queue:
                emit_pv(j, ets.pop(j))
            for j in dve_tiles:
                emit_pv(j, ets.pop(j))
            osb = spool.tile([128, QW], BF16, tag="osb")
            nc.vector.tensor_copy(osb[:], psum_o[:])
            st["partial"], st["osb"] = partial, osb

        def emit_finalize(b, st):
            partial, osb = st["partial"], st["osb"]
            # den: partial^T via XBAR, then free-dim reduce -> [128, 8] f32
            rpt = spool.tile([128, QW], BF16, tag="rpt")
            nc.sync.dma_start_transpose(
                out=rpt[:].rearrange("p (j k) -> p j k", j=8, k=128),
                in_=partial[:],
            )
            den_t = spool.tile([128, 8], F32, tag="dent")
            nc.vector.tensor_reduce(
                den_t[:],
                rpt[:].rearrange("p (j k) -> p j k", j=8, k=128),
                axis=mybir.AxisListType.X, op=mybir.AluOpType.add,
            )
            recip = spool.tile([128, 8], F32, tag="recip")
            nc.vector.reciprocal(recip[:], den_t[:])

            # assemble full 2KB output rows: XBAR transpose then 4x bf16
            # scale; the f32 cast happens on the output DMA.
            ot = spool.tile([128, QW], BF16, tag="ot")
            nc.sync.dma_start_transpose(
                out=ot[:].rearrange("p (j d) -> p j d", j=8, d=D),
                in_=osb[:],
            )
            for tt in range(2):
                of = spool.tile([128, G * D], BF16, tag="outf")
                for h in range(G):
                    j = h * 2 + tt
                    nc.gpsimd.tensor_scalar(
                        out=of[:, h * D : (h + 1) * D],
                        in0=ot[:, j * 128 : (j + 1) * 128],
                        scalar1=recip[:, j : j + 1],
                        scalar2=None, op0=mybir.AluOpType.mult,
                    )
                nc.gpsimd.dma_start(
                    out=out_ext[b * Q + tt * 128 : b * Q + (tt + 1) * 128, :],
                    in_=of[:],
                )

        for _rep in range(repeat if variant != "noop" else 0):
            btwsb = idxpool.tile([128, B * 8], I16, tag="btwsb")
            nc.sync.dma_start(out=btwsb[:], in_=btw_ext[:, :])
            st = {}
            st[0] = emit_prep_dma(0, btwsb)
            st[1] = {"kvt": st[0]["kvt"]}
            emit_prep_compute(0, st[0], pair_side=0)
            st[1]["vtbp"] = st[0]["vtbp"]
            for b in range(B):
                if b == 1:
                    st[2] = emit_prep_dma(2, btwsb)
                    st[3] = {"kvt": st[2]["kvt"]}
                if b - 1 >= 0:
                    fb = b - 1
                    hook = (lambda fb=fb: (emit_finalize(fb, st[fb]),
                                           st.pop(fb)))
                else:
                    hook = None
                emit_compute(b, st[b], mid_hook=hook)
                if b + 1 < B:
                    emit_prep_compute(b + 1, st[b + 1],
                                      pair_side=(b + 1) % 2)
                    if (b + 1) % 2 == 0:
                        st[b + 2]["vtbp"] = st[b + 1]["vtbp"]
            emit_finalize(B - 1, st[B - 1])

    nc.finalize()
    return nc, consts_arrays


def make_consts():
    idb = np.eye(128).astype(ml_dtypes.bfloat16)
    return dict(idb=idb)


def shard_inputs(q, k, v, kv_cache, slot_mapping, block_tables, seq_lens,
                 query_start_loc, mask_arrays):
    """mask_arrays carries the qp/c0s/mask consts from build_nc (name kept
    for test.py compatibility)."""
    consts = make_consts()
    kv_cache = np.asarray(kv_cache)
    block_tables = np.asarray(block_tables)
    # Host pre-scatter of the new k/v tokens into the cache copy.
    kc_all = np.asarray(kv_cache[0]).reshape(NBLK * BLOCK, 8, D).copy()
    vc_all = np.asarray(kv_cache[1]).reshape(NBLK * BLOCK, 8, D).copy()
    sm = np.asarray(slot_mapping).reshape(-1)
    kc_all[sm] = np.asarray(k).reshape(-1, 8, D)
    vc_all[sm] = np.asarray(v).reshape(-1, 8, D)
    kc_all = kc_all.reshape(NBLK, BLOCK, 8, D)
    vc_all = vc_all.reshape(NBLK, BLOCK, 8, D)
    # gather index tile [128, B*8]: wrapped in 16 partitions, replicated
    btw = np.zeros((128, B * 8), np.int16)
    for b in range(B):
        bt = np.asarray(block_tables[b]).astype(np.int16)
        for p in range(128):
            for c in range(8):
                btw[p, b * 8 + c] = bt[c * 16 + p % 16]
    in_maps = []
    for h in range(8):
        kvc = np.stack([
            np.ascontiguousarray(kc_all[:, :, h, :]).reshape(NBLK, BLOCK * D),
            np.ascontiguousarray(vc_all[:, :, h, :]).reshape(NBLK, BLOCK * D),
        ])
        m = {
            "q": np.ascontiguousarray(np.asarray(q)[:, h * G * D : (h + 1) * G * D]),
            "kvc": kvc,
            "btw": btw,
            **consts,
            **mask_arrays,
        }
        in_maps.append(m)
    return in_maps


def assemble_output(results):
    return np.concatenate([results[h]["out"] for h in range(8)], axis=1)


# ---------------------------------------------------------------------------
# Harness entry point: kernel(**inputs) with FULL (unsharded) inputs.
# ---------------------------------------------------------------------------
from concourse.bass_utils import run_bass_kernel_spmd

_CACHE = {}


def _get_nc(seq_lens):
    key = tuple(int(x) for x in seq_lens)
    if key not in _CACHE:
        _CACHE[key] = build_nc(key)
    return _CACHE[key]


def kernel(q, k, v, kv_cache, slot_mapping, block_tables, seq_lens,
           query_start_loc, **extra):
    q = np.asarray(q); k = np.asarray(k); v = np.asarray(v)
    kv_cache = np.asarray(kv_cache)
    slot_mapping = np.asarray(slot_mapping)
    block_tables = np.asarray(block_tables)
    seq_lens = np.asarray(seq_lens)
    nc, mask_arrays = _get_nc(seq_lens)
    in_maps = shard_inputs(q, k, v, kv_cache, slot_mapping, block_tables,
                           seq_lens, query_start_loc, mask_arrays)
    res = run_bass_kernel_spmd(nc, in_maps, core_ids=list(range(8)))
    return assemble_output(res.results)


# revision 10
# speedup vs baseline: 1.4663x; 1.4663x over previous
"""Paged GQA attention Bass kernel for TRN2, SPMD over 8 cores (v6).

Sharding: tensor-parallel over KV heads. Core h owns KV head h and its 4
query heads. Per-core: B=4 seqs x S=2048 kv x (4 heads * 256 q) x d=128.

v6 design (HW-measured instruction rates; see work/micro.py):
  - host pre-scatters the new k/v tokens into the cache copy, so there is
    no device tail path. One pair-gather per 2 seqs covers all 128 real
    blocks: 16 slot-tiles per seq, kv position = 16j + c (permuted), no
    pad blobs, no den correction.
  - exp on ACT (616ns/tile measured, no bias needed for bf16 range);
    optionally a few tiles per seq on DVE via the custom ops EXP_SEED_ANT
    (2nd-order seed of exp(x*SCALE/64), 650ns) and EXP_FIN_MASK_ANT
    (^64 + causal compare vs qp, 635ns).
  - causal masks are dense bf16 [128,1024] consts (rows j<112 are ones),
    applied with tensor_mul (199ns measured); accumulation is
    tensor_copy/tensor_add (199ns). scalar_tensor_tensor measured 3.5x
    slower than tensor_tensor on HW (cost model is wrong there) - v5's
    mistake.
  - den: partial^T via one XBAR transpose -> tensor_reduce (axis X) ->
    [128, 8] f32 -> reciprocal. No PE ones-matmul, no den transposes.
  - output: psum_o -> osb bf16 (DVE) -> XBAR -> 4x bf16 scale ops -> f32
    cast on the output DMA (gpsimd).
  - PV stays bf16 (fp8 et/V measured 3.5e-2..6.8e-2 rel err vs the 2e-2
    gate). PV matmuls for DVE-exp'd tiles are deferred to the end of the
    seq's PE stream so PE never waits on the slower DVE exp.
  PE is the bottleneck: 32768 cycles/seq (scores+PV) ~ 54.6us/rep.
"""
import numpy as np
import ml_dtypes

import concourse.bass as bass
import concourse.bacc as bacc
import concourse.mybir as mybir
from concourse.tile import TileContext

F32 = mybir.dt.float32
BF16 = mybir.dt.bfloat16
I16 = mybir.dt.int16

B, Q, S = 4, 256, 2048
G, D = 4, 128
BLOCK = 16
NBLK = 640               # cache pool blocks
NT = 16                  # kv tiles per seq (slot-tiles)
QW = G * Q               # 1024
SCALE = float(D) ** -0.5
EXP_N = 64               # seed^64: 6 squarings
DVE_TILES_DEFAULT = ()


# ---------------------------------------------------------------------------
# Custom DVE exp ops (registered into concourse.dve_ops on first use).
# ---------------------------------------------------------------------------
_EXP_OPS = {}


def _register_exp_ops():
    if _EXP_OPS:
        return _EXP_OPS
    import concourse.dve_ops as dve_ops
    from concourse.dve_spec import (
        Spec, Src0, Src1, C0, C1, C2, sq, lower, _has_src1,
    )
    from concourse.dve_uop import DveOpSpec

    def _seed_ref(in0, in1, s0, s1, imm2):
        return (imm2 * (in0.astype(np.float32) * s0 + s1) ** 2
                + imm2).astype(np.float32)

    seed_spec = Spec(
        body=sq(Src0 * C0 + C1) * C2 + C2,
        reference=_seed_ref,
    )

    x = Src0
    for _ in range(6):
        x = sq(x)
    fin_spec = Spec(
        body=x * (Src1 >= C0),
        reference=lambda in0, in1, s0, s1, imm2: (
            (in0.astype(np.float32) ** 64)
            * (in1.astype(np.float32) >= s0)
        ).astype(np.float32),
    )

    for name, spec in (("EXP_SEED_ANT", seed_spec),
                       ("EXP_FIN_MASK_ANT", fin_spec)):
        if name not in dve_ops._SUB_OPCODE_FOR_NAME:
            row = max(dve_ops._SUB_OPCODE_FOR_NAME.values()) + 1
            assert row < 0x20, "custom DVE opcode rows exhausted"
            dve_ops._SUB_OPCODE_FOR_NAME[name] = row
        row = dve_ops._SUB_OPCODE_FOR_NAME[name]
        shas = {}
        for ver in ("v3", "v4"):
            uops = lower(spec, ver=ver)
            shas[ver] = DveOpSpec(
                name=name, opcode=row, uops=uops, rd1_en=_has_src1(spec)
            ).sha(ver)
        op = dve_ops.DveOp(name, spec, subdim=False, uops_sha=shas)
        if all(o.name != name for o in dve_ops.OPS):
            dve_ops.OPS.append(op)
        dve_ops.CUSTOM_DVE_SPECS[name] = spec
        _EXP_OPS[name] = op
    return _EXP_OPS


def build_consts(seq_lens):
    """qp [128, QW] bf16: t*128+k per column (exact in bf16), used by the
    custom DVE mask compare. c0s [128, B*NT] f32: threshold
    16j + c - (sl_b - 256). masks: dense bf16 [128, QW] visibility tiles
    (deduped); needs[(b, c)] -> mask name or None (all-visible)."""
    col = np.arange(QW)
    tk = (col % Q).astype(np.float64)
    qp = np.broadcast_to(tk[None, :], (128, QW)).astype(ml_dtypes.bfloat16)
    j = np.arange(128)
    c0s = np.zeros((128, B * NT), np.float32)
    mask_arrays, needs, cache = {}, {}, {}
    for b in range(B):
        sl = int(seq_lens[b])
        qpos = sl - Q + (col % Q)
        for c in range(NT):
            c0s[:, b * NT + c] = 16 * j + c - (sl - Q)
            kpos = 16 * j + c
            vis = kpos[:, None] <= qpos[None, :]
            if vis.all():
                needs[(b, c)] = None
                continue
            key = vis.tobytes()
            if key not in cache:
                name = f"mask{len(cache)}"
                cache[key] = name
                mask_arrays[name] = vis.astype(ml_dtypes.bfloat16)
            needs[(b, c)] = cache[key]
    return {"qp": np.ascontiguousarray(qp), "c0s": c0s, **mask_arrays}, needs


def build_nc(seq_lens=(2048,) * B, variant="full", repeat=1,
             dve_tiles=DVE_TILES_DEFAULT):
    exp_ops = _register_exp_ops()
    nc = bacc.Bacc(None, target_bir_lowering=False, debug=False)

    consts_arrays, mask_needs = build_consts(seq_lens)

    q_ext = nc.declare_dram_parameter("q", [B * Q, G * D], F32, isOutput=False)
    kvc_ext = nc.declare_dram_parameter("kvc", [2, NBLK, BLOCK * D], F32,
                                        isOutput=False)
    btw_ext = nc.declare_dram_parameter("btw", [128, B * 8], I16, isOutput=False)
    idb_ext = nc.declare_dram_parameter("idb", [128, 128], BF16, isOutput=False)
    qp_ext = nc.declare_dram_parameter("qp", [128, QW], BF16, isOutput=False)
    c0s_ext = nc.declare_dram_parameter("c0s", [128, B * NT], F32,
                                        isOutput=False)
    mask_ext = {
        name: nc.declare_dram_parameter(name, [128, QW], BF16, isOutput=False)
        for name in consts_arrays if name.startswith("mask")
    }

    out_ext = nc.declare_dram_parameter("out", [B * Q, G * D], F32, isOutput=True)

    if variant == "nodve":
        dve_tiles = ()
    dve_tiles = tuple(dve_tiles)

    from contextlib import ExitStack

    with TileContext(nc) as tc, ExitStack() as stack:
        cpool = stack.enter_context(tc.tile_pool(name="consts", bufs=1))
        dpool = stack.enter_context(tc.tile_pool(name="dram", bufs=1, space="DRAM"))
        kvpool = stack.enter_context(tc.tile_pool(name="kvp", bufs=2))
        spool = stack.enter_context(tc.tile_pool(name="sbuf", bufs=3))
        idxpool = stack.enter_context(tc.tile_pool(name="idxp", bufs=2))
        et_pool = stack.enter_context(tc.tile_pool(name="et", bufs=8))
        sd_pool = stack.enter_context(tc.tile_pool(name="sd", bufs=2))
        n_psc = 3 if not dve_tiles else 2
        ppool_sc = stack.enter_context(tc.tile_pool(name="psc", bufs=n_psc, space="PSUM"))
        if dve_tiles:
            ppool_dve = stack.enter_context(tc.tile_pool(name="pdve", bufs=1, space="PSUM"))
        ppool_o = stack.enter_context(tc.tile_pool(name="po", bufs=1, space="PSUM"))

        # ---- constants ----
        idb = cpool.tile([128, 128], BF16, tag="idb")
        nc.sync.dma_start(out=idb[:], in_=idb_ext[:, :])
        qp = cpool.tile([128, QW], BF16, tag="qp")
        nc.sync.dma_start(out=qp[:], in_=qp_ext[:, :])
        c0s = cpool.tile([128, B * NT], F32, tag="c0s")
        nc.sync.dma_start(out=c0s[:], in_=c0s_ext[:, :])
        masks = {}
        for name in mask_ext:
            m = cpool.tile([128, QW], BF16, tag=name)
            nc.sync.dma_start(out=m[:], in_=mask_ext[name][:, :])
            masks[name] = m

        if variant == "noop":
            z = spool.tile([128, 128], F32, tag="outsb")
            nc.vector.memset(z[:], 0.0)
            nc.sync.dma_start(out=out_ext[0:128, 0:128], in_=z[:])

        # PE clock warm-up (HAM gate holds PE at 1.2 GHz until ~3.4us busy).
        if variant != "noop":
            for _w in range(28):
                warm = ppool_sc.tile([128, 128], F32, tag="psc", name="warm")
                nc.tensor.matmul(warm[:], lhsT=idb[:], rhs=idb[:],
                                 start=True, stop=True)

        # ---- one-time staging ----
        # kvb blob tensor: [K block (16x128) | V block] per block, bf16.
        kvb = dpool.tile([NBLK, 2 * BLOCK * D], BF16, tag="kvb")
        nc.gpsimd.dma_start(
            out=kvb[:, :].rearrange("b (k e) -> k b e", k=2, e=BLOCK * D),
            in_=kvc_ext[:, :, :],
        )
        # q staged bf16: [p=tok%128, r=tok//128, (h d)]
        qcb = cpool.tile([128, (B * Q // 128) * G * D], BF16, tag="qcb")
        qcb_v = qcb[:].rearrange("p (r hd) -> p r hd", r=B * Q // 128, hd=G * D)
        nc.gpsimd.dma_start(
            out=qcb_v[:, :, :],
            in_=q_ext.rearrange("(r p) hd -> p r hd", p=128),
        )

        def emit_prep_dma(b, btwsb):
            """Pair gather for seqs (b, b+1): 256 idxs, kvt [128, 32, 256]."""
            st = {}
            nj = 256
            st["kvt"] = kvpool.tile([128, 32 * nj], BF16, tag="kvt", name="kvt")
            nc.gpsimd.dma_gather(
                out_ap=st["kvt"][:].rearrange("p (c j) -> p c j", c=32, j=nj),
                in_ap=kvb[:, :],
                idxs_ap=btwsb[:, b * 8 : b * 8 + nj // 16],
                num_idxs=nj, num_idxs_reg=nj, elem_size=2 * BLOCK * D,
                transpose=True, single_packet=False,
            )
            return st

        def emit_prep_compute(b, st, pair_side):
            nj = 256
            kvt_v = st["kvt"][:].rearrange("p (c j) -> p c j", c=32, j=nj)
            sd = pair_side
            if sd == 0:
                vtbp = kvpool.tile([128, 2 * NT * D], BF16, tag="vtb")
                nc.sync.dma_start_transpose(
                    out=vtbp[:].rearrange("p (m d) -> p m d", m=2 * NT, d=D),
                    in_=st["kvt"][:, 16 * nj : 32 * nj],
                )
                st["vtbp"] = vtbp
            vtb_v = st["vtbp"][:].rearrange(
                "p (c s d) -> p c s d", c=NT, s=2, d=D)
            st["kt_tiles"] = [kvt_v[:, i, sd * 128 : (sd + 1) * 128]
                              for i in range(NT)]
            st["v_tiles"] = [vtb_v[:, i, sd, :] for i in range(NT)]
            # q^T: [128 d, (h, t, tok)] via 2 XBAR transposes
            qt_t = spool.tile([128, QW], BF16, tag="qt")
            qt_v = qt_t[:].rearrange("p (h t k) -> p h t k", h=G, t=2, k=128)
            for t in range(2):
                nc.sync.dma_start_transpose(
                    out=qt_v[:, :, t, :],
                    in_=qcb_v[:, 2 * b + t, :],
                )
            st["qt"] = qt_t

        def emit_compute(b, st, mid_hook=None):
            partial = spool.tile([128, QW], BF16, tag="partial")
            psum_o = ppool_o.tile([128, QW], F32, tag="po")
            qt_t = st["qt"]
            kt_tiles, v_tiles = st["kt_tiles"], st["v_tiles"]

            act_tiles = [i for i in range(NT) if i not in dve_tiles]
            # scores emission order: first DVE tile up front, the rest
            # spread every 4 ACT tiles (dedicated psum pool, ring of 1)
            sc_order = []
            dq = list(dve_tiles)
            ai = 0
            while dq or ai < len(act_tiles):
                if dq and (not sc_order or len(sc_order) % 5 == 0):
                    sc_order.append(dq.pop(0))
                elif ai < len(act_tiles):
                    sc_order.append(act_tiles[ai]); ai += 1
                else:
                    sc_order.append(dq.pop(0))
            pv_order = act_tiles + list(dve_tiles)
            pv_last_tile = pv_order[-1]

            def emit_pv(i, et):
                if variant == "nopv":
                    return
                v_tile = v_tiles[i]
                for half in range(2):
                    nc.tensor.matmul(
                        psum_o[:, half * 512 : (half + 1) * 512],
                        lhsT=v_tile,
                        rhs=et[:, half * 512 : (half + 1) * 512],
                        start=(i == pv_order[0]), stop=(i == pv_last_tile),
                    )

            ets = {}
            pv_queue = []
            acc_first = True
            n_emitted = 0
            for i in sc_order:
                if n_emitted == 6 and mid_hook is not None:
                    mid_hook()
                n_emitted += 1
                pool = ppool_dve if i in dve_tiles else ppool_sc
                psc = pool.tile([128, QW], F32, tag="psc", name="psc")
                for half in range(2):
                    nc.tensor.matmul(
                        psc[:, half * 512 : (half + 1) * 512],
                        lhsT=kt_tiles[i],
                        rhs=qt_t[:, half * 512 : (half + 1) * 512],
                        start=True, stop=True,
                    )
                et = et_pool.tile([128, QW], BF16, tag="et")
                if i in dve_tiles:
                    c0 = c0s[:, b * NT + i : b * NT + i + 1]
                    sd1 = sd_pool.tile([128, QW], F32, tag="sd")
                    nc.vector._custom_dve(
                        exp_ops["EXP_SEED_ANT"], out=sd1[:], in0=psc[:],
                        s0=SCALE / EXP_N, s1=1.0, imm2=0.5,
                    )
                    nc.vector._custom_dve(
                        exp_ops["EXP_FIN_MASK_ANT"], out=et[:], in0=sd1[:],
                        in1=qp[:], s0=c0,
                    )
                    ets[i] = et
                    continue
                if variant == "noexp":
                    nc.scalar.activation(
                        et[:, 0:128], psc[:, 0:128],
                        mybir.ActivationFunctionType.Exp, scale=SCALE,
                    )
                else:
                    nc.scalar.activation(
                        et[:], psc[:], mybir.ActivationFunctionType.Exp,
                        scale=SCALE,
                    )
                mname = mask_needs[(b, i)]
                if mname is not None and variant != "nomask":
                    nc.vector.tensor_mul(et[:], et[:], masks[mname][:])
                if variant != "noacc":
                    if acc_first:
                        nc.vector.tensor_copy(partial[:], et[:])
                        acc_first = False
                    else:
                        nc.vector.tensor_add(partial[:], partial[:], et[:])
                ets[i] = et
                pv_queue.append(i)
                if len(pv_queue) > 1:
                    j = pv_queue.pop(0)
                    emit_pv(j, ets.pop(j))
            for j in pv_queue:
                emit_pv(j, ets.pop(j))
            for j in dve_tiles:
                if variant != "noacc":
                    nc.vector.tensor_add(partial[:], partial[:], ets[j][:])
                emit_pv(j, ets.pop(j))
            osb = spool.tile([128, QW], BF16, tag="osb")
            nc.vector.tensor_copy(osb[:], psum_o[:])
            st["partial"], st["osb"] = partial, osb

        def emit_finalize(b, st):
            partial, osb = st["partial"], st["osb"]
            # den: partial^T via XBAR, then free-dim reduce -> [128, 8] f32
            rpt = spool.tile([128, QW], BF16, tag="rpt")
            nc.sync.dma_start_transpose(
                out=rpt[:].rearrange("p (j k) -> p j k", j=8, k=128),
                in_=partial[:],
            )
            den_t = spool.tile([128, 8], F32, tag="dent")
            nc.vector.tensor_reduce(
                den_t[:],
                rpt[:].rearrange("p (j k) -> p j k", j=8, k=128),
                axis=mybir.AxisListType.X, op=mybir.AluOpType.add,
            )
            recip = spool.tile([128, 8], F32, tag="recip")
            nc.vector.reciprocal(recip[:], den_t[:])

            # assemble full 2KB output rows: XBAR transpose then 4x bf16
            # scale; the f32 cast happens on the output DMA.
            ot = spool.tile([128, QW], BF16, tag="ot")
            nc.sync.dma_start_transpose(
                out=ot[:].rearrange("p (j d) -> p j d", j=8, d=D),
                in_=osb[:],
            )
            for tt in range(2):
                of = spool.tile([128, G * D], BF16, tag="outf")
                for h in range(G):
                    j = h * 2 + tt
                    nc.gpsimd.tensor_scalar(
                        out=of[:, h * D : (h + 1) * D],
                        in0=ot[:, j * 128 : (j + 1) * 128],
                        scalar1=recip[:, j : j + 1],
                        scalar2=None, op0=mybir.AluOpType.mult,
                    )
                nc.gpsimd.dma_start(
                    out=out_ext[b * Q + tt * 128 : b * Q + (tt + 1) * 128, :],
                    in_=of[:],
                )

        for _rep in range(repeat if variant != "noop" else 0):
            btwsb = idxpool.tile([128, B * 8], I16, tag="btwsb")
            nc.sync.dma_start(out=btwsb[:], in_=btw_ext[:, :])
            st = {}
            st[0] = emit_prep_dma(0, btwsb)
            st[1] = {"kvt": st[0]["kvt"]}
            emit_prep_compute(0, st[0], pair_side=0)
            st[1]["vtbp"] = st[0]["vtbp"]
            for b in range(B):
                if b == 1:
                    st[2] = emit_prep_dma(2, btwsb)
                    st[3] = {"kvt": st[2]["kvt"]}
                if b - 1 >= 0:
                    fb = b - 1
                    hook = (lambda fb=fb: (emit_finalize(fb, st[fb]),
                                           st.pop(fb)))
                else:
                    hook = None
                emit_compute(b, st[b], mid_hook=hook)
                if b + 1 < B:
                    emit_prep_compute(b + 1, st[b + 1],
                                      pair_side=(b + 1) % 2)
                    if (b + 1) % 2 == 0:
                        st[b + 2]["vtbp"] = st[b + 1]["vtbp"]
            emit_finalize(B - 1, st[B - 1])

    nc.finalize()
    return nc, consts_arrays


def make_consts():
    idb = np.eye(128).astype(ml_dtypes.bfloat16)
    return dict(idb=idb)


def shard_inputs(q, k, v, kv_cache, slot_mapping, block_tables, seq_lens,
                 query_start_loc, mask_arrays):
    """mask_arrays carries the qp/c0s/mask consts from build_nc (name kept
    for test.py compatibility)."""
    consts = make_consts()
    kv_cache = np.asarray(kv_cache)
    block_tables = np.asarray(block_tables)
    # Host pre-scatter of the new k/v tokens into the cache copy.
    kc_all = np.asarray(kv_cache[0]).reshape(NBLK * BLOCK, 8, D).copy()
    vc_all = np.asarray(kv_cache[1]).reshape(NBLK * BLOCK, 8, D).copy()
    sm = np.asarray(slot_mapping).reshape(-1)
    kc_all[sm] = np.asarray(k).reshape(-1, 8, D)
    vc_all[sm] = np.asarray(v).reshape(-1, 8, D)
    kc_all = kc_all.reshape(NBLK, BLOCK, 8, D)
    vc_all = vc_all.reshape(NBLK, BLOCK, 8, D)
    # gather index tile [128, B*8]: wrapped in 16 partitions, replicated
    btw = np.zeros((128, B * 8), np.int16)
    for b in range(B):
        bt = np.asarray(block_tables[b]).astype(np.int16)
        for p in range(128):
            for c in range(8):
                btw[p, b * 8 + c] = bt[c * 16 + p % 16]
    in_maps = []
    for h in range(8):
        kvc = np.stack([
            np.ascontiguousarray(kc_all[:, :, h, :]).reshape(NBLK, BLOCK * D),
            np.ascontiguousarray(vc_all[:, :, h, :]).reshape(NBLK, BLOCK * D),
        ])
        m = {
            "q": np.ascontiguousarray(np.asarray(q)[:, h * G * D : (h + 1) * G * D]),
            "kvc": kvc,
            "btw": btw,
            **consts,
            **mask_arrays,
        }
        in_maps.append(m)
    return in_maps


def assemble_output(results):
    return np.concatenate([results[h]["out"] for h in range(8)], axis=1)


# ---------------------------------------------------------------------------
# Harness entry point: kernel(**inputs) with FULL (unsharded) inputs.
# ---------------------------------------------------------------------------
from concourse.bass_utils import run_bass_kernel_spmd

_CACHE = {}


def _get_nc(seq_lens):
    key = tuple(int(x) for x in seq_lens)
    if key not in _CACHE:
        _CACHE[key] = build_nc(key)
    return _CACHE[key]


def kernel(q, k, v, kv_cache, slot_mapping, block_tables, seq_lens,
           query_start_loc, **extra):
    q = np.asarray(q); k = np.asarray(k); v = np.asarray(v)
    kv_cache = np.asarray(kv_cache)
    slot_mapping = np.asarray(slot_mapping)
    block_tables = np.asarray(block_tables)
    seq_lens = np.asarray(seq_lens)
    nc, mask_arrays = _get_nc(seq_lens)
    in_maps = shard_inputs(q, k, v, kv_cache, slot_mapping, block_tables,
                           seq_lens, query_start_loc, mask_arrays)
    res = run_bass_kernel_spmd(nc, in_maps, core_ids=list(range(8)))
    return assemble_output(res.results)
